# revision 1
# baseline (speedup 1.0000x reference)
"""GPS layer (GraphGPS) Trainium2 kernel: ResGatedGraphConv + dense per-graph MHA + FFN.

Sharding: data-parallel over the 64 graphs (8 graphs / 4096 nodes per core).
Edges are sorted by destination and bucketed into 128-node windows per core;
message aggregation uses one-hot matmuls accumulating in PSUM per window.
q/v rows (keyed by random src) are fetched via indirect DMA from a
device-computed [N, 256] q|v table in DRAM.
"""
import sys
sys.path.insert(0, '/opt/trn_rl_repo')
import numpy as np
import concourse.bass as bass
import concourse.bacc as bacc
import concourse.tile as tile
from concourse import mybir
from concourse.bass_utils import run_bass_kernel_spmd
from concourse.masks import make_identity

N, E, D, H, B, S = 32768, 524288, 128, 4, 64, 512
HD = D // H          # 32
NC = 8               # cores
NPC = N // NC        # 4096 nodes per core
GPC = B // NC        # 8 graphs per core
WIN = 128            # dst window
NWIN = NPC // WIN    # 32 windows per core
EPS = 1e-5
F32 = mybir.dt.float32
I32 = mybir.dt.int32


def _prep_edges(edge_index):
    src = np.asarray(edge_index[0], dtype=np.int64)
    dst = np.asarray(edge_index[1], dtype=np.int64)
    order = np.argsort(dst, kind='stable')
    ss, ds = src[order], dst[order]
    wid = ds // WIN                       # global window id, 0..255
    counts = np.bincount(wid, minlength=NC * NWIN)
    tpw = int(np.ceil(counts.max() / 128))   # tiles per window (uniform)
    cap = tpw * 128
    gsrc = np.zeros((NC * NWIN, cap), np.int32)
    ldst = np.full((NC * NWIN, cap), -1.0, np.float32)
    offs = np.zeros(NC * NWIN + 1, np.int64)
    np.cumsum(counts, out=offs[1:])
    for w in range(NC * NWIN):
        s, e = offs[w], offs[w + 1]
        n = e - s
        gsrc[w, :n] = ss[s:e]
        ldst[w, :n] = (ds[s:e] - w * WIN).astype(np.float32)
    # per-core [NWIN, tpw, 128] with edge p of tile t on partition p
    gsrc = gsrc.reshape(NC, NWIN, tpw, 128)
    ldst = ldst.reshape(NC, NWIN, tpw, 128)
    return gsrc, ldst, tpw


def _build(nc, tpw):
    P = 128
    xT = nc.declare_dram_parameter("xT", [P, N], F32, isOutput=False)
    xT_loc = nc.declare_dram_parameter("xT_loc", [P, NPC], F32, isOutput=False)
    gsrc = nc.declare_dram_parameter("gsrc", [NWIN, tpw, 128], I32, isOutput=False)
    ldst = nc.declare_dram_parameter("ldst", [NWIN, tpw, 128], F32, isOutput=False)
    WqT = nc.declare_dram_parameter("WqT", [P, P], F32, isOutput=False)
    WvT = nc.declare_dram_parameter("WvT", [P, P], F32, isOutput=False)
    WkT = nc.declare_dram_parameter("WkT", [P, P], F32, isOutput=False)
    WresT = nc.declare_dram_parameter("WresT", [P, P], F32, isOutput=False)
    WskipT = nc.declare_dram_parameter("WskipT", [P, P], F32, isOutput=False)
    ipwT = nc.declare_dram_parameter("ipwT", [P, 3 * P], F32, isOutput=False)
    opwT = nc.declare_dram_parameter("opwT", [P, P], F32, isOutput=False)
    W1T = nc.declare_dram_parameter("W1T", [P, 2 * P], F32, isOutput=False)
    W2T = nc.declare_dram_parameter("W2T", [2 * P, P], F32, isOutput=False)
    # column vectors [128, 1]: biases and folded-BN scale/shift
    cols = nc.declare_dram_parameter("cols", [P, 16], F32, isOutput=False)
    # rows on partition 0: kq bias (bk+bq), v bias
    rows = nc.declare_dram_parameter("rows", [1, 2 * P], F32, isOutput=False)
    ipb = nc.declare_dram_parameter("ipb", [P, 3], F32, isOutput=False)
    b1c = nc.declare_dram_parameter("b1c", [P, 2], F32, isOutput=False)
    qv_dram = nc.dram_tensor("qv_table", [N, 2 * P], F32)
    outT = nc.declare_dram_parameter("outT", [P, NPC], F32, isOutput=True)

    ctx = nc  # alias
    with tile.TileContext(nc) as tc:
        import contextlib
        with contextlib.ExitStack() as es:
            one = es.enter_context(tc.tile_pool(name="one", bufs=1))
            sb = es.enter_context(tc.tile_pool(name="sb", bufs=2))
            ps = es.enter_context(tc.tile_pool(name="ps", bufs=2, space="PSUM"))
            psA = es.enter_context(tc.tile_pool(name="psA", bufs=1, space="PSUM"))

            ident = one.tile([P, P], F32)
            make_identity(nc, ident[:])
            # small weights resident in SBUF
            wq = one.tile([P, P], F32); nc.sync.dma_start(out=wq[:], in_=WqT[:])
            wv = one.tile([P, P], F32); nc.sync.dma_start(out=wv[:], in_=WvT[:])
            wk = one.tile([P, P], F32); nc.sync.dma_start(out=wk[:], in_=WkT[:])
            wres = one.tile([P, P], F32); nc.sync.dma_start(out=wres[:], in_=WresT[:])
            wskip = one.tile([P, P], F32); nc.sync.dma_start(out=wskip[:], in_=WskipT[:])
            wip = one.tile([P, 3 * P], F32); nc.sync.dma_start(out=wip[:], in_=ipwT[:])
            wop = one.tile([P, P], F32); nc.sync.dma_start(out=wop[:], in_=opwT[:])
            w1 = one.tile([P, 2 * P], F32); nc.sync.dma_start(out=w1[:], in_=W1T[:])
            w2a = one.tile([P, P], F32); nc.sync.dma_start(out=w2a[:], in_=W2T[:P])
            w2b = one.tile([P, P], F32); nc.sync.dma_start(out=w2b[:], in_=W2T[P:])
            colv = one.tile([P, 16], F32); nc.sync.dma_start(out=colv[:], in_=cols[:])
            rowv = one.tile([1, 2 * P], F32); nc.sync.dma_start(out=rowv[:], in_=rows[:])
            ipbv = one.tile([P, 3], F32); nc.sync.dma_start(out=ipbv[:], in_=ipb[:])
            b1v = one.tile([P, 2], F32); nc.sync.dma_start(out=b1v[:], in_=b1c[:])
            onesc = one.tile([1, P], F32); nc.vector.memset(onesc[:], 1.0)
            iota_r = one.tile([P, P], I32)
            nc.gpsimd.iota(iota_r[:], pattern=[[1, P]], base=0, channel_multiplier=0)
            iota_f = one.tile([P, P], F32)
            nc.vector.tensor_copy(iota_f[:], iota_r[:])
            stackI = one.tile([P, HD], F32)
            nc.vector.tensor_tensor(
                out=stackI[:], in0=colv[:, 10:11].to_broadcast([P, HD]),
                in1=iota_f[:, :HD], op=mybir.AluOpType.is_equal)
            # kq/v bias broadcast tiles [128, 256]: cols 0:128 = bk+bq, 128:256 = bv
            bias_ps = ps.tile([P, 2 * P], F32, tag="s256")
            nc.tensor.matmul(bias_ps[:], lhsT=onesc[:], rhs=rowv[:], start=True, stop=True)
            kqb = one.tile([P, P], F32)
            nc.vector.tensor_copy(kqb[:], bias_ps[:, :P])
            vb2 = one.tile([P, 2 * P], F32)
            nc.vector.tensor_copy(vb2[:], bias_ps[:])
            nc.vector.memset(vb2[:, :P], 0.0)

            # ---- phase 1a: qv table [N, 256] -> DRAM ----
            for blk in range(8):           # xT in [128, 4096] chunks
                xc = sb.tile([P, 4096], F32, tag="xc")
                nc.sync.dma_start(out=xc[:], in_=xT[:, blk * 4096:(blk + 1) * 4096])
                for c in range(32):        # node chunks of 128
                    pt = ps.tile([P, 2 * P], F32, tag="s256")
                    st = xc[:, c * P:(c + 1) * P]
                    nc.tensor.matmul(pt[:, :P], lhsT=st, rhs=wq[:], start=True, stop=True)
                    nc.tensor.matmul(pt[:, P:], lhsT=st, rhs=wv[:], start=True, stop=True)
                    qvt = sb.tile([P, 2 * P], F32, tag="qvsb")
                    nc.vector.tensor_add(out=qvt[:], in0=pt[:], in1=vb2[:])
                    nc.gpsimd.dma_start(
                        out=qv_dram[(blk * 32 + c) * P:(blk * 32 + c + 1) * P, :],
                        in_=qvt[:])

            # ---- phase 1b: local tables (dim-major h_in1T, skipT; node-major k) ----
            hin1 = one.tile([P, NPC], F32)   # dim-major relu(x@WresT+bres)
            skip = one.tile([P, NPC], F32)
            ktab = one.tile([P, NWIN * P], F32)  # node-major k per window
            xl = one.tile([P, NPC], F32)
            nc.sync.dma_start(out=xl[:], in_=xT_loc[:])
            for c in range(8):
                sl = slice(c * 512, (c + 1) * 512)
                pr = ps.tile([P, 512], F32, tag="b512")
                nc.tensor.matmul(pr[:], lhsT=wres[:], rhs=xl[:, sl], start=True, stop=True)
                nc.scalar.activation(hin1[:, sl], pr[:],
                                     mybir.ActivationFunctionType.Relu,
                                     bias=colv[:, 0:1], scale=1.0)
                pr2 = ps.tile([P, 512], F32, tag="b512")
                nc.tensor.matmul(pr2[:], lhsT=wskip[:], rhs=xl[:, sl], start=True, stop=True)
                nc.scalar.activation(skip[:, sl], pr2[:],
                                     mybir.ActivationFunctionType.Identity,
                                     bias=colv[:, 1:2], scale=1.0)
            for w in range(NWIN):
                pk = ps.tile([P, P], F32, tag="s256")
                nc.tensor.matmul(pk[:], lhsT=xl[:, w * P:(w + 1) * P], rhs=wk[:],
                                 start=True, stop=True)
                nc.vector.tensor_add(out=ktab[:, w * P:(w + 1) * P], in0=pk[:], in1=kqb[:])

            # ---- phase 2: message passing ----
            hloc = one.tile([P, NPC], F32)   # dim-major h_local after BN1l
            for w in range(NWIN):
                ldw = sb.tile([P, tpw], F32, tag="ldw")
                nc.sync.dma_start(out=ldw[:], in_=ldst[w].rearrange("t p -> p t"))
                gsw = sb.tile([P, tpw], I32, tag="gsw")
                nc.sync.dma_start(out=gsw[:], in_=gsrc[w].rearrange("t p -> p t"))
                # one-hot block [e, n] for all tiles of this window
                obig = sb.tile([P, tpw * P], F32, tag="obig")
                nc.vector.tensor_tensor(
                    out=obig[:].rearrange("p (t n) -> p t n", t=tpw),
                    in0=ldw[:, :, None].to_broadcast([P, tpw, P]),
                    in1=iota_f[:, None, :].to_broadcast([P, tpw, P]),
                    op=mybir.AluOpType.is_equal)
                agg = psA.tile([P, P], F32, tag="agg")
                kwin = ktab[:, w * P:(w + 1) * P]
                for t in range(tpw):
                    qvg = sb.tile([P, 2 * P], F32, tag="qvg")
                    nc.gpsimd.indirect_dma_start(
                        out=qvg[:], out_offset=None, in_=qv_dram[:],
                        in_offset=bass.IndirectOffsetOnAxis(ap=gsw[:, t:t + 1], axis=0))
                    osl = obig[:, t * P:(t + 1) * P]
                    pot = ps.tile([P, P], F32, tag="s256")
                    nc.tensor.transpose(out=pot[:], in_=osl, identity=ident[:])
                    ot = sb.tile([P, P], F32, tag="ot")
                    nc.vector.tensor_copy(ot[:], pot[:])
                    parg = ps.tile([P, P], F32, tag="s256")
                    nc.tensor.matmul(parg[:], lhsT=ot[:], rhs=kwin, start=True, stop=False)
                    nc.tensor.matmul(parg[:], lhsT=ident[:], rhs=qvg[:, :P],
                                     start=False, stop=True)
                    sig = sb.tile([P, P], F32, tag="sig")
                    nc.scalar.activation(sig[:], parg[:],
                                         mybir.ActivationFunctionType.Sigmoid)
                    msg = sb.tile([P, P], F32, tag="msg")
                    nc.vector.tensor_mul(out=msg[:], in0=sig[:], in1=qvg[:, P:])
                    nc.tensor.matmul(agg[:], lhsT=osl, rhs=msg[:],
                                     start=(t == 0), stop=(t == tpw - 1))
                # agg [n, d] -> transpose to dim-major, combine + BN1l
                asb = sb.tile([P, P], F32, tag="asb")
                nc.vector.tensor_copy(asb[:], agg[:])
                paT = ps.tile([P, P], F32, tag="s256")
                nc.tensor.transpose(out=paT[:], in_=asb[:], identity=ident[:])
                wsl = slice(w * P, (w + 1) * P)
                t1 = sb.tile([P, P], F32, tag="t1")
                nc.vector.tensor_add(out=t1[:], in0=paT[:], in1=skip[:, wsl])
                nc.vector.tensor_add(out=t1[:], in0=t1[:], in1=hin1[:, wsl])
                nc.vector.tensor_scalar(out=hloc[:, wsl], in0=t1[:],
                                        scalar1=colv[:, 2:3], scalar2=colv[:, 3:4],
                                        op0=mybir.AluOpType.mult,
                                        op1=mybir.AluOpType.add)

            # ---- phase 3: attention + FFN per graph ----
            for g in range(GPC):
                gs = slice(g * S, (g + 1) * S)
                hg = hin1[:, gs]
                qkv = []
                for j in range(3):
                    qkvj = sb.tile([P, S], F32, tag=f"qkv{j}")
                    qkv.append(qkvj)
                for j in range(3):
                    pq = ps.tile([P, S], F32, tag="b512")
                    nc.tensor.matmul(pq[:], lhsT=wip[:, j * P:(j + 1) * P], rhs=hg,
                                     start=True, stop=True)
                    nc.scalar.activation(qkv[j][:], pq[:],
                                         mybir.ActivationFunctionType.Identity,
                                         bias=ipbv[:, j:j + 1], scale=1.0)
                ctxg = sb.tile([P, S], F32, tag="ctxg")
                for h in range(H):
                    qh = sb.tile([HD, S], F32, tag="qh")
                    nc.vector.tensor_copy(qh[:], qkv[0][h * HD:(h + 1) * HD, :])
                    kh = sb.tile([HD, S], F32, tag="kh")
                    nc.vector.tensor_copy(kh[:], qkv[1][h * HD:(h + 1) * HD, :])
                    vh = sb.tile([HD, S], F32, tag="vh")
                    nc.vector.tensor_copy(vh[:], qkv[2][h * HD:(h + 1) * HD, :])
                    # vh node-major [S, HD] + ones col -> [S, HD+1]
                    vaug = sb.tile([P, 4 * (HD + 1)], F32, tag="vaug")
                    for c in range(4):
                        pvT = ps.tile([P, HD], F32, tag="s256")
                        nc.tensor.transpose(out=pvT[:], in_=vh[:, c * P:(c + 1) * P],
                                            identity=ident[:HD, :HD])
                        nc.vector.tensor_copy(vaug[:, c * (HD + 1):c * (HD + 1) + HD], pvT[:])
                        nc.vector.memset(vaug[:, c * (HD + 1) + HD:(c + 1) * (HD + 1)], 1.0)
                    pctx = ps.tile([HD + 1, S], F32, tag="b512")
                    for c in range(4):
                        psc = ps.tile([P, S], F32, tag="b512")
                        nc.tensor.matmul(psc[:], lhsT=kh[:, c * P:(c + 1) * P], rhs=qh,
                                         start=True, stop=True)
                        esc = sb.tile([P, S], F32, tag="esc")
                        nc.scalar.activation(esc[:], psc[:],
                                             mybir.ActivationFunctionType.Exp,
                                             scale=float(1.0 / np.sqrt(HD)))
                        nc.tensor.matmul(pctx[:], lhsT=vaug[:, c * (HD + 1):(c + 1) * (HD + 1)],
                                         rhs=esc[:], start=(c == 0), stop=(c == 3))
                    den = sb.tile([1, S], F32, tag="den")
                    nc.vector.reciprocal(den[:], pctx[HD:HD + 1, :])
                    denb = sb.tile([HD, S], F32, tag="denb")
                    nc.gpsimd.partition_broadcast(denb[:], den[:])
                    nc.vector.tensor_mul(out=ctxg[h * HD:(h + 1) * HD, :],
                                         in0=pctx[:HD, :], in1=denb[:])
                # out proj + BN1a combine with h_in1, then FFN + BN2
                pop = ps.tile([P, S], F32, tag="b512")
                nc.tensor.matmul(pop[:], lhsT=wop[:], rhs=ctxg[:], start=True, stop=True)
                hat = sb.tile([P, S], F32, tag="hat")
                nc.scalar.activation(hat[:], pop[:],
                                     mybir.ActivationFunctionType.Identity,
                                     bias=colv[:, 4:5], scale=1.0)
                nc.vector.tensor_add(out=hat[:], in0=hat[:], in1=hg)
                nc.vector.tensor_scalar(out=hat[:], in0=hat[:],
                                        scalar1=colv[:, 5:6], scalar2=colv[:, 6:7],
                                        op0=mybir.AluOpType.mult,
                                        op1=mybir.AluOpType.add)
                nc.vector.tensor_add(out=hat[:], in0=hat[:], in1=hloc[:, gs])
                ff = []
                for c in range(2):
                    ffc = sb.tile([P, S], F32, tag=f"ff{c}")
                    ff.append(ffc)
                for c in range(2):
                    pf = ps.tile([P, S], F32, tag="b512")
                    nc.tensor.matmul(pf[:], lhsT=w1[:, c * P:(c + 1) * P], rhs=hat[:],
                                     start=True, stop=True)
                    nc.scalar.activation(ff[c][:], pf[:],
                                         mybir.ActivationFunctionType.Relu,
                                         bias=b1v[:, c:c + 1], scale=1.0)
                pf2 = ps.tile([P, S], F32, tag="b512")
                nc.tensor.matmul(pf2[:], lhsT=w2a[:], rhs=ff[0][:], start=True, stop=False)
                nc.tensor.matmul(pf2[:], lhsT=w2b[:], rhs=ff[1][:], start=False, stop=True)
                ot2 = sb.tile([P, S], F32, tag="ot2")
                nc.scalar.activation(ot2[:], pf2[:],
                                     mybir.ActivationFunctionType.Identity,
                                     bias=colv[:, 7:8], scale=1.0)
                nc.vector.tensor_add(out=ot2[:], in0=ot2[:], in1=hat[:])
                nc.vector.tensor_scalar(out=ot2[:], in0=ot2[:],
                                        scalar1=colv[:, 8:9], scalar2=colv[:, 9:10],
                                        op0=mybir.AluOpType.mult,
                                        op1=mybir.AluOpType.add)
                nc.gpsimd.dma_start(out=outT[:, gs], in_=ot2[:])
    nc.compile()
    return nc


def kernel(x, edge_index, batch_ids, Wres, bres, Wk, bk, Wq, bq, Wv, bv,
           Wskip, bskip, g1l, b1l, g1a, b1a, in_proj_w, in_proj_b,
           out_proj_w, out_proj_b, W1, b1, W2, b2, g2, b2g):
    x = np.asarray(x, dtype=np.float32)
    gsrc, ldst, tpw = _prep_edges(np.asarray(edge_index))
    xT = np.ascontiguousarray(x.T)
    bnf = 1.0 / np.sqrt(1.0 + EPS)
    cols = np.zeros((128, 16), np.float32)
    cols[:, 0] = bres; cols[:, 1] = bskip
    cols[:, 2] = g1l * bnf; cols[:, 3] = b1l
    cols[:, 4] = out_proj_b
    cols[:, 5] = g1a * bnf; cols[:, 6] = b1a
    cols[:, 7] = b2; cols[:, 8] = g2 * bnf; cols[:, 9] = b2g
    cols[:, 10] = np.arange(128) % 32
    rows = np.concatenate([bk + bq, bv]).reshape(1, 256).astype(np.float32)
    common = dict(
        xT=xT,
        WqT=np.ascontiguousarray(Wq.T), WvT=np.ascontiguousarray(Wv.T),
        WkT=np.ascontiguousarray(Wk.T), WresT=np.ascontiguousarray(Wres.T),
        WskipT=np.ascontiguousarray(Wskip.T),
        ipwT=np.ascontiguousarray(in_proj_w.T), opwT=np.ascontiguousarray(out_proj_w.T),
        W1T=np.ascontiguousarray(W1.T), W2T=np.ascontiguousarray(W2.T),
        cols=cols, rows=rows,
        ipb=np.ascontiguousarray(np.asarray(in_proj_b, np.float32).reshape(3, 128).T),
        b1c=np.ascontiguousarray(np.asarray(b1, np.float32).reshape(2, 128).T),
    )
    in_maps = []
    for c in range(NC):
        m = dict(common)
        m["xT_loc"] = np.ascontiguousarray(xT[:, c * NPC:(c + 1) * NPC])
        m["gsrc"] = np.ascontiguousarray(gsrc[c])
        m["ldst"] = np.ascontiguousarray(ldst[c])
        in_maps.append(m)

    nc = bacc.Bacc("TRN2", target_bir_lowering=False, debug=False, num_devices=NC)
    _build(nc, tpw)
    res = run_bass_kernel_spmd(nc, in_maps, list(range(NC)))
    if getattr(res, "exec_time_ns", None):
        print(f"HW exec time: {res.exec_time_ns} ns")
    out = np.empty((N, D), np.float32)
    for c in range(NC):
        out[c * NPC:(c + 1) * NPC] = res.results[c]["outT"].T
    return out

